# revision 5
# baseline (speedup 1.0000x reference)
"""Trainium2 Bass kernel v2 for nn_AAFM (sparse attention with distance decay).

Math (per batch b):
    q = query @ Wq.T + bq ; k = key @ Wk.T (bk cancels) ; v = value @ Wv.T
    expA = exp(-alpha*log2(N)*dist)             [n_q, n_k]
    num  = expA @ (exp(k) * v)                  [n_q, d]
    den  = expA @ exp(k)                        [n_q, d]
    out  = sigmoid(q) * (num/den + bv)

Design (v2, ~105us vs 135us v1 baseline):
  - TRANSPOSED main matmul: out^T[d, q] = (ekv|ek stationary).T @ expA moving.
    Stationaries are [128,128] slices reused across q-halves and LDWEIGHTS
    fully hides behind 512-col streams: MMs run at ~215-226ns (vs 258-286 in
    the [q,d]-oriented v1 where the expA stationary changed every MM).
  - All-bf16 datapath. fp8 (incl. DoubleRow den) was tried and REJECTED: the
    output is a weighted mean of near-zero-mean v, so any iid per-element
    perturbation of the weight chain (expA, ek, or k-proj inputs) hits the
    output at the full per-element quantization rms (~2% for e4m3) -- no
    sqrt(n) averaging, no num/den cancellation (only per-d column factors
    cancel, e.g. bk).
  - Host pre-casts dist/q/k/v to bf16: HBM traffic 30MB -> 16MB per core
    (memory floor ~45us, well under the ~85us PE floor; no DGE casts needed,
    so all steady-state loads ride the cheap sync ring, one descriptor per
    tensor -- ring sequencers pay ~0.7us per dma_start).
  - bq applied as per-partition ACT bias inside the tanh (d is the partition
    dim in the transposed layout): kills v1's separate bias pass. bv added
    per-partition on DVE via tensor_scalar_add (NEVER gpsimd tensor_scalar:
    measured 7.5us per [128,512] op there).
  - sigmoid(q) = 0.5*(tanh(q/2)+1), 0.5 folded into Wv/bv host-side (tanh
    shares the exp ACT table set; sigmoid would force ~1.5us table reloads).
  - expA produced by ACT exp directly from bf16 dist tiles (bf16-in runs
    ~1.9us per 256K elems vs 2.3us f32-in).
  - Output stored transposed [d, q] bf16; host re-transposes + upcasts.
  - Data-parallel over batch: 32 batches -> 8 cores x 4, no collectives.
  - Prologue: wk/kT/wv/vT ride the gpsimd DGE ring (issues from t=0; sync
    ring needs ~6us queue bootstrap); PE warm-up MMs bridge the DMA lead-in
    so the HAM clock governor holds 2.4GHz into the first projections.
"""

import sys

for _p in ("/opt/trn_rl_repo",):
    if _p not in sys.path:
        sys.path.append(_p)

import ml_dtypes
import numpy as np

import concourse.bass as bass
import concourse.tile as tile
from concourse import bacc, mybir
from concourse.bass_utils import run_bass_kernel_spmd

N_CORES = 8
B = 32
N = 1024  # graph size
D = 256  # d_model
BPC = B // N_CORES  # batches per core
KT = N // 128  # 8 k-tiles of 128 tokens
F32 = mybir.dt.float32
BF16 = mybir.dt.bfloat16
Alu = mybir.AluOpType
Act = mybir.ActivationFunctionType


def build_graph(c_coef: float):
    nc = bacc.Bacc(
        "TRN2", target_bir_lowering=False, debug=False, num_devices=N_CORES
    )

    distT = nc.declare_dram_parameter("distT", [BPC, N, N], BF16, isOutput=False)
    qT = nc.declare_dram_parameter("qT", [BPC, D, N], BF16, isOutput=False)
    kT = nc.declare_dram_parameter("kT", [BPC, D, N], BF16, isOutput=False)
    vT = nc.declare_dram_parameter("vT", [BPC, D, N], BF16, isOutput=False)
    WqT = nc.declare_dram_parameter("WqT", [D, D], BF16, isOutput=False)
    WkT = nc.declare_dram_parameter("WkT", [D, D], BF16, isOutput=False)
    WvT = nc.declare_dram_parameter("WvT", [D, D], BF16, isOutput=False)
    bq2_d = nc.declare_dram_parameter("bq2", [128, 2], F32, isOutput=False)
    bv2_d = nc.declare_dram_parameter("bv2", [128, 2], F32, isOutput=False)
    # out^T: [bi*2 + d_chunk, 128 d, 1024 q]
    out_d = nc.declare_dram_parameter("out", [BPC * 2, 128, N], BF16, isOutput=True)

    with tile.TileContext(nc) as tc:
        with (
            tc.tile_pool(name="const", bufs=1) as const_p,
            tc.tile_pool(name="qkv", bufs=2) as qkv_p,
            tc.tile_pool(name="dist", bufs=3) as dist_p,
            tc.tile_pool(name="stage", bufs=2) as stage_p,
            tc.tile_pool(name="eps", bufs=2) as eps_p,
            tc.tile_pool(name="outst", bufs=2) as out_p,
            tc.tile_pool(name="kvp", bufs=3, space="PSUM") as kvp,
            tc.tile_pool(name="qpp", bufs=1, space="PSUM") as qpp,
            tc.tile_pool(name="nump", bufs=2, space="PSUM") as nump,
            tc.tile_pool(name="denp", bufs=2, space="PSUM") as denp,
        ):
            # ---- constants (once) ----
            # wk/wv ride the gpsimd ring: it issues from t=0 while the sync
            # ring needs ~6us of queue bootstrap; the first kv-proj only
            # needs wk/wv + kT/vT.
            w_tiles = {}
            for nm, wd in (("wk", WkT), ("wv", WvT), ("wq", WqT)):
                w_tiles[nm] = const_p.tile([128, 2, D], BF16, tag=nm, name=nm)
            nc.gpsimd.dma_start(
                w_tiles["wk"][:], WkT[:].rearrange("(j p) e -> p j e", p=128)
            )
            nc.sync.dma_start(
                w_tiles["wq"][:], WqT[:].rearrange("(j p) e -> p j e", p=128)
            )
            bq2_t = const_p.tile([128, 2], F32, tag="bq2")
            nc.sync.dma_start(bq2_t[:], bq2_d[:])
            bv2_t = const_p.tile([128, 2], F32, tag="bv2")
            nc.sync.dma_start(bv2_t[:], bv2_d[:])
            # PE warm-up while first DMAs land (HAM clock governor)
            warm_sb = const_p.tile([128, 512], BF16, tag="warm")
            nc.vector.memset(warm_sb[:], 0.0)
            warm_ps = qpp.tile([128, 512], F32, tag="qp")

            def warm(n, cols=512):
                for _ in range(n):
                    nc.tensor.matmul(
                        warm_ps[:, 0:cols], warm_sb[:, 0:128],
                        warm_sb[:, 0:cols], start=True, stop=True,
                    )

            warm(8)
            warm(8, cols=128)

            def phase_load(bi, first=False):
                # One descriptor per tensor (ring sequencers pay ~0.7us per
                # dma_start). Batch 0: kT/vT ride the gpsimd ring (starts at
                # t=0); everything else syncs in priority order.
                xt = {}
                if first:
                    # gpsimd ring order after wk: kT, wv, vT (earliest proj)
                    for nm, xd in (("kT", kT), ("vT", vT)):
                        t = qkv_p.tile([128, 2, N], BF16, tag=nm)
                        nc.gpsimd.dma_start(
                            t[:], xd[bi].rearrange("(j p) n -> p j n", p=128)
                        )
                        xt[nm] = t
                        if nm == "kT":
                            nc.gpsimd.dma_start(
                                w_tiles["wv"][:],
                                WvT[:].rearrange("(j p) e -> p j e", p=128),
                            )
                    t = qkv_p.tile([128, 2, N], BF16, tag="qT")
                    nc.sync.dma_start(
                        t[:], qT[bi].rearrange("(j p) n -> p j n", p=128)
                    )
                    xt["qT"] = t
                else:
                    for nm, xd in (("kT", kT), ("vT", vT), ("qT", qT)):
                        t = qkv_p.tile([128, 2, N], BF16, tag=nm)
                        nc.sync.dma_start(
                            t[:], xd[bi].rearrange("(j p) n -> p j n", p=128)
                        )
                        xt[nm] = t
                dists = []
                for c in range(2):
                    dt_t = dist_p.tile([128, 4, N], BF16, tag="dist")
                    nc.sync.dma_start(
                        dt_t[:],
                        distT[bi, c * 512 : (c + 1) * 512, :].rearrange(
                            "(j p) q -> p j q", p=128
                        ),
                    )
                    dists.append(dt_t)
                return xt, dists

            def alloc_stage():
                return {
                    "expA": stage_p.tile([128, KT, N], BF16, tag="expA",
                                         name="expA"),
                    "ekv": stage_p.tile([128, KT, D], BF16, tag="ekv",
                                        name="ekv"),
                    "ek8": stage_p.tile([128, KT, D], BF16, tag="ek8",
                                        name="ek8"),
                    "tanh": stage_p.tile([128, 2, N], BF16, tag="tanh",
                                         name="tanh_t"),
                }

            def proj_tile(p, xt, t0):
                """kv projections + ek/ekv for one 128-token tile."""
                cols = slice(t0 * 128, (t0 + 1) * 128)
                kv = kvp.tile([128, 2, D], F32, tag="kv")
                for dt in range(2):
                    nc.tensor.matmul(
                        kv[:, 0, :],
                        xt["kT"][:, dt, cols],
                        w_tiles["wk"][:, dt, :],
                        start=(dt == 0),
                        stop=(dt == 1),
                    )
                for dt in range(2):
                    nc.tensor.matmul(
                        kv[:, 1, :],
                        xt["vT"][:, dt, cols],
                        w_tiles["wv"][:, dt, :],
                        start=(dt == 0),
                        stop=(dt == 1),
                    )
                nc.scalar.activation(p["ek8"][:, t0, :], kv[:, 0, :], Act.Exp)
                nc.vector.scalar_tensor_tensor(
                    p["ekv"][:, t0, :],
                    kv[:, 1, :],
                    1.0,
                    p["ek8"][:, t0, :],
                    Alu.mult,
                    Alu.mult,
                )

            def qproj_piece(p, xt, i):
                """q projection + tanh for (d_chunk, q_half) = divmod(i, 2)."""
                c, h = divmod(i, 2)
                hs = slice(h * 512, (h + 1) * 512)
                qp = qpp.tile([128, 512], F32, tag="qp")
                for j in range(2):
                    nc.tensor.matmul(
                        qp[:],
                        w_tiles["wq"][:, j, c * 128 : (c + 1) * 128],
                        xt["qT"][:, j, hs],
                        start=(j == 0),
                        stop=(j == 1),
                    )
                nc.scalar.activation(
                    p["tanh"][:, c, hs], qp[:], Act.Tanh,
                    bias=bq2_t[:, c : c + 1], scale=0.5,
                )

            def expa_chunk(p, dists, c):
                nc.scalar.activation(
                    p["expA"][:, c * 2 : (c + 1) * 2, :],
                    dists[c // 2][:, (c % 2) * 2 : (c % 2) * 2 + 2, :],
                    Act.Exp,
                    scale=-c_coef,
                )

            def main_unit(bi, p, i, out_t):
                """num/den/post for (d_chunk, q_half) = divmod(i, 2)."""
                c, h = divmod(i, 2)
                cs = slice(c * 128, (c + 1) * 128)
                hs = slice(h * 512, (h + 1) * 512)
                dn = denp.tile([128, 512], F32, tag="dn")
                for k in range(KT):
                    nc.tensor.matmul(
                        dn[:],
                        p["ek8"][:, k, cs],
                        p["expA"][:, k, hs],
                        start=(k == 0),
                        stop=(k == KT - 1),
                    )
                nm = nump.tile([128, 512], F32, tag="nm")
                for k in range(KT):
                    nc.tensor.matmul(
                        nm[:],
                        p["ekv"][:, k, cs],
                        p["expA"][:, k, hs],
                        start=(k == 0),
                        stop=(k == KT - 1),
                    )
                r = eps_p.tile([128, 512], F32, tag="r")
                nc.vector.reciprocal_approx_fast(r[:], dn[:])
                m = eps_p.tile([128, 512], F32, tag="m")
                a = eps_p.tile([128, 512], F32, tag="a")
                for u in range(2):
                    us = slice(u * 256, (u + 1) * 256)
                    uo = slice(h * 512 + u * 256, h * 512 + (u + 1) * 256)
                    nc.vector.tensor_mul(m[:, us], nm[:, us], r[:, us])
                    nc.vector.tensor_scalar_add(
                        a[:, us], m[:, us], bv2_t[:, c : c + 1]
                    )
                    nc.vector.scalar_tensor_tensor(
                        out_t[:, c, uo], p["tanh"][:, c, uo], 1.0, a[:, us],
                        Alu.add, Alu.mult,
                    )

            def phase_main(bi, p, nxt):
                """main(b) interleaved with stage-prep(b+1)."""
                p2 = alloc_stage() if nxt else None
                out_t = out_p.tile([128, 2, N], BF16, tag="outst")
                for i in range(4):
                    main_unit(bi, p, i, out_t)
                    if nxt is not None:
                        proj_tile(p2, nxt[0], 2 * i)
                        proj_tile(p2, nxt[0], 2 * i + 1)
                        qproj_piece(p2, nxt[0], i)
                        if i < 3:
                            expa_chunk(p2, nxt[1], i)
                    # store each finished half right away: shorter exit tail
                    c, h = divmod(i, 2)
                    nc.sync.dma_start(
                        out_d[bi * 2 + c][:, h * 512 : (h + 1) * 512],
                        out_t[:, c, h * 512 : (h + 1) * 512],
                    )
                if nxt is not None:
                    expa_chunk(p2, nxt[1], 3)
                return p2

            # ---- prologue: batch 0 ----
            xt0, dists0 = phase_load(0, first=True)
            staged = alloc_stage()
            for t0 in range(KT):
                proj_tile(staged, xt0, t0)
                warm(4, cols=128)
                if t0 % 2 == 1:
                    qproj_piece(staged, xt0, t0 // 2)
                    expa_chunk(staged, dists0, t0 // 2)
            for bi in range(BPC):
                ld = phase_load(bi + 1) if bi + 1 < BPC else None
                staged = phase_main(bi, staged, ld)

    nc.compile()
    return nc


def prepare_in_maps(inputs: dict):
    query = np.asarray(inputs["query"], dtype=np.float32)
    key_ = np.asarray(inputs["key_"], dtype=np.float32)
    value = np.asarray(inputs["value"], dtype=np.float32)
    dist = np.asarray(inputs["dist"], dtype=np.float32)
    Wq = np.asarray(inputs["Wq"], dtype=np.float32)
    Wk = np.asarray(inputs["Wk"], dtype=np.float32)
    Wv = np.asarray(inputs["Wv"], dtype=np.float32)
    bq = np.asarray(inputs["bq"], dtype=np.float32)
    bv = np.asarray(inputs["bv"], dtype=np.float32)
    alpha_raw = np.asarray(inputs["alpha_raw"], dtype=np.float64)

    alpha = float(np.logaddexp(0.0, alpha_raw)) + 1e-6  # softplus + eps
    c_coef = float(alpha * np.log2(float(N)))

    bf = ml_dtypes.bfloat16
    distT = np.ascontiguousarray(dist.transpose(0, 2, 1)).astype(bf)
    qT = np.ascontiguousarray(query.transpose(0, 2, 1)).astype(bf)
    kT = np.ascontiguousarray(key_.transpose(0, 2, 1)).astype(bf)
    vT = np.ascontiguousarray(value.transpose(0, 2, 1)).astype(bf)
    WqT = np.ascontiguousarray(Wq.T).astype(bf)
    WkT = np.ascontiguousarray(Wk.T).astype(bf)
    WvT = (np.ascontiguousarray(Wv.T) * 0.5).astype(bf)  # fold sigmoid's 0.5
    # per-partition bias columns: [128 d, 2 chunks]; 0.5 folds sigmoid scale
    bq2 = np.ascontiguousarray((0.5 * bq).reshape(2, 128).T).astype(np.float32)
    bv2 = np.ascontiguousarray((0.5 * bv).reshape(2, 128).T).astype(np.float32)

    in_maps = []
    for i in range(N_CORES):
        s = slice(i * BPC, (i + 1) * BPC)
        in_maps.append(
            {
                "distT": distT[s],
                "qT": qT[s],
                "kT": kT[s],
                "vT": vT[s],
                "WqT": WqT,
                "WkT": WkT,
                "WvT": WvT,
                "bq2": bq2,
                "bv2": bv2,
            }
        )
    return in_maps, c_coef


def run_sharded(inputs: dict, trace: bool = False):
    """Returns (full_output [32,1024,256] f32, BassKernelResults)."""
    in_maps, c_coef = prepare_in_maps(inputs)
    nc = build_graph(c_coef)
    res = run_bass_kernel_spmd(
        nc, in_maps, core_ids=list(range(N_CORES)), trace=trace
    )
    # out: per core [BPC*2, 128, N] -> [BPC, 2, 128, N] -> [BPC, N, 256]
    parts = []
    for i in range(N_CORES):
        o = np.asarray(res.results[i]["out"]).astype(np.float32)
        o = o.reshape(BPC, 2, 128, N).reshape(BPC, D, N).transpose(0, 2, 1)
        parts.append(o)
    out = np.ascontiguousarray(np.concatenate(parts, axis=0))
    return out, res


def kernel(**inputs) -> np.ndarray:
    try:
        out, _ = run_sharded(inputs, trace=False)
    except Exception:
        # one retry: transient NRT device errors (wedged core) recover on rerun
        out, _ = run_sharded(inputs, trace=False)
    return out


# revision 6
# speedup vs baseline: 1.0239x; 1.0239x over previous
"""Trainium2 Bass kernel v2 for nn_AAFM (sparse attention with distance decay).

Math (per batch b):
    q = query @ Wq.T + bq ; k = key @ Wk.T (bk cancels) ; v = value @ Wv.T
    expA = exp(-alpha*log2(N)*dist)             [n_q, n_k]
    num  = expA @ (exp(k) * v)                  [n_q, d]
    den  = expA @ exp(k)                        [n_q, d]
    out  = sigmoid(q) * (num/den + bv)

Design (v2, ~105us vs 135us v1 baseline):
  - TRANSPOSED main matmul: out^T[d, q] = (ekv|ek stationary).T @ expA moving.
    Stationaries are [128,128] slices reused across q-halves and LDWEIGHTS
    fully hides behind 512-col streams: MMs run at ~215-226ns (vs 258-286 in
    the [q,d]-oriented v1 where the expA stationary changed every MM).
  - All-bf16 datapath. fp8 (incl. DoubleRow den) was tried and REJECTED: the
    output is a weighted mean of near-zero-mean v, so any iid per-element
    perturbation of the weight chain (expA, ek, or k-proj inputs) hits the
    output at the full per-element quantization rms (~2% for e4m3) -- no
    sqrt(n) averaging, no num/den cancellation (only per-d column factors
    cancel, e.g. bk).
  - Host pre-casts dist/q/k/v to bf16: HBM traffic 30MB -> 16MB per core
    (memory floor ~45us, well under the ~85us PE floor; no DGE casts needed,
    so all steady-state loads ride the cheap sync ring, one descriptor per
    tensor -- ring sequencers pay ~0.7us per dma_start).
  - bq applied as per-partition ACT bias inside the tanh (d is the partition
    dim in the transposed layout): kills v1's separate bias pass. bv added
    per-partition on DVE via tensor_scalar_add (NEVER gpsimd tensor_scalar:
    measured 7.5us per [128,512] op there).
  - sigmoid(q) = 0.5*(tanh(q/2)+1), 0.5 folded into Wv/bv host-side (tanh
    shares the exp ACT table set; sigmoid would force ~1.5us table reloads).
  - expA produced by ACT exp directly from bf16 dist tiles (bf16-in runs
    ~1.9us per 256K elems vs 2.3us f32-in).
  - Output stored transposed [d, q] bf16; host re-transposes + upcasts.
  - Data-parallel over batch: 32 batches -> 8 cores x 4, no collectives.
  - Prologue: wk/kT/wv/vT ride the gpsimd DGE ring (issues from t=0; sync
    ring needs ~6us queue bootstrap); PE warm-up MMs bridge the DMA lead-in
    so the HAM clock governor holds 2.4GHz into the first projections.
"""

import sys

for _p in ("/opt/trn_rl_repo",):
    if _p not in sys.path:
        sys.path.append(_p)

import ml_dtypes
import numpy as np

import concourse.bass as bass
import concourse.tile as tile
from concourse import bacc, mybir
from concourse.bass_utils import run_bass_kernel_spmd

N_CORES = 8
B = 32
N = 1024  # graph size
D = 256  # d_model
BPC = B // N_CORES  # batches per core
KT = N // 128  # 8 k-tiles of 128 tokens
F32 = mybir.dt.float32
BF16 = mybir.dt.bfloat16
Alu = mybir.AluOpType
Act = mybir.ActivationFunctionType


def build_graph(c_coef: float):
    nc = bacc.Bacc(
        "TRN2", target_bir_lowering=False, debug=False, num_devices=N_CORES
    )

    distT = nc.declare_dram_parameter("distT", [BPC, N, N], BF16, isOutput=False)
    qT = nc.declare_dram_parameter("qT", [BPC, D, N], BF16, isOutput=False)
    kT = nc.declare_dram_parameter("kT", [BPC, D, N], BF16, isOutput=False)
    vT = nc.declare_dram_parameter("vT", [BPC, D, N], BF16, isOutput=False)
    WqT = nc.declare_dram_parameter("WqT", [D, D], BF16, isOutput=False)
    WkT = nc.declare_dram_parameter("WkT", [D, D], BF16, isOutput=False)
    WvT = nc.declare_dram_parameter("WvT", [D, D], BF16, isOutput=False)
    bq2_d = nc.declare_dram_parameter("bq2", [128, 2], F32, isOutput=False)
    bv2_d = nc.declare_dram_parameter("bv2", [128, 2], F32, isOutput=False)
    # out^T: [bi*2 + d_chunk, 128 d, 1024 q]
    out_d = nc.declare_dram_parameter("out", [BPC * 2, 128, N], BF16, isOutput=True)

    with tile.TileContext(nc) as tc:
        with (
            tc.tile_pool(name="const", bufs=1) as const_p,
            tc.tile_pool(name="qkv", bufs=2) as qkv_p,
            tc.tile_pool(name="dist", bufs=6) as dist_p,
            tc.tile_pool(name="stage", bufs=2) as stage_p,
            tc.tile_pool(name="eps", bufs=2) as eps_p,
            tc.tile_pool(name="outst", bufs=2) as out_p,
            tc.tile_pool(name="kvp", bufs=3, space="PSUM") as kvp,
            tc.tile_pool(name="qpp", bufs=1, space="PSUM") as qpp,
            tc.tile_pool(name="nump", bufs=2, space="PSUM") as nump,
            tc.tile_pool(name="denp", bufs=2, space="PSUM") as denp,
        ):
            # ---- constants (once) ----
            # wk/wv ride the gpsimd ring: it issues from t=0 while the sync
            # ring needs ~6us of queue bootstrap; the first kv-proj only
            # needs wk/wv + kT/vT.
            w_tiles = {}
            for nm, wd in (("wk", WkT), ("wv", WvT), ("wq", WqT)):
                w_tiles[nm] = const_p.tile([128, 2, D], BF16, tag=nm, name=nm)
            nc.gpsimd.dma_start(
                w_tiles["wk"][:], WkT[:].rearrange("(j p) e -> p j e", p=128)
            )
            nc.gpsimd.dma_start(
                w_tiles["wv"][:], WvT[:].rearrange("(j p) e -> p j e", p=128)
            )
            nc.sync.dma_start(
                w_tiles["wq"][:], WqT[:].rearrange("(j p) e -> p j e", p=128)
            )
            bq2_t = const_p.tile([128, 2], F32, tag="bq2")
            nc.sync.dma_start(bq2_t[:], bq2_d[:])
            bv2_t = const_p.tile([128, 2], F32, tag="bv2")
            nc.sync.dma_start(bv2_t[:], bv2_d[:])
            # PE warm-up while first DMAs land (HAM clock governor)
            warm_sb = const_p.tile([128, 512], BF16, tag="warm")
            nc.vector.memset(warm_sb[:], 0.0)
            warm_ps = qpp.tile([128, 512], F32, tag="qp")

            def warm(n, cols=512):
                for _ in range(n):
                    nc.tensor.matmul(
                        warm_ps[:, 0:cols], warm_sb[:, 0:128],
                        warm_sb[:, 0:cols], start=True, stop=True,
                    )

            warm(8)
            warm(8, cols=128)

            def phase_load(bi, first=False):
                # One descriptor per tensor (ring sequencers pay ~0.7us per
                # dma_start). Batch 0: kT/vT ride the gpsimd ring (starts at
                # t=0); everything else syncs in priority order.
                xt = {}
                if first:
                    for nm, xd in (("kT", kT), ("vT", vT)):
                        t = qkv_p.tile([128, 2, N], BF16, tag=nm)
                        nc.sync.dma_start(
                            t[:], xd[bi].rearrange("(j p) n -> p j n", p=128)
                        )
                        xt[nm] = t
                    dists = []
                    # first two dist quarters outrank qT on the sync ring:
                    # the serial ACT exp chain gates main(0)
                    for c in range(2):
                        dt_t = dist_p.tile([128, 2, N], BF16, tag="dist")
                        nc.sync.dma_start(
                            dt_t[:],
                            distT[bi, c * 256 : (c + 1) * 256, :].rearrange(
                                "(j p) q -> p j q", p=128
                            ),
                        )
                        dists.append(dt_t)
                    t = qkv_p.tile([128, 2, N], BF16, tag="qT")
                    nc.sync.dma_start(
                        t[:], qT[bi].rearrange("(j p) n -> p j n", p=128)
                    )
                    xt["qT"] = t
                    for c in range(2, 4):
                        dt_t = dist_p.tile([128, 2, N], BF16, tag="dist")
                        nc.sync.dma_start(
                            dt_t[:],
                            distT[bi, c * 256 : (c + 1) * 256, :].rearrange(
                                "(j p) q -> p j q", p=128
                            ),
                        )
                        dists.append(dt_t)
                    return xt, dists
                for nm, xd in (("kT", kT), ("vT", vT), ("qT", qT)):
                    t = qkv_p.tile([128, 2, N], BF16, tag=nm)
                    nc.sync.dma_start(
                        t[:], xd[bi].rearrange("(j p) n -> p j n", p=128)
                    )
                    xt[nm] = t
                dists = []
                for c in range(4):
                    dt_t = dist_p.tile([128, 2, N], BF16, tag="dist")
                    nc.sync.dma_start(
                        dt_t[:],
                        distT[bi, c * 256 : (c + 1) * 256, :].rearrange(
                            "(j p) q -> p j q", p=128
                        ),
                    )
                    dists.append(dt_t)
                return xt, dists

            def alloc_stage():
                return {
                    "expA": stage_p.tile([128, KT, N], BF16, tag="expA",
                                         name="expA"),
                    "ekv": stage_p.tile([128, KT, D], BF16, tag="ekv",
                                        name="ekv"),
                    "ek8": stage_p.tile([128, KT, D], BF16, tag="ek8",
                                        name="ek8"),
                    "tanh": stage_p.tile([128, 2, N], BF16, tag="tanh",
                                         name="tanh_t"),
                }

            def proj_tile(p, xt, t0):
                """kv projections + ek/ekv for one 128-token tile."""
                cols = slice(t0 * 128, (t0 + 1) * 128)
                kv = kvp.tile([128, 2, D], F32, tag="kv")
                for dt in range(2):
                    nc.tensor.matmul(
                        kv[:, 0, :],
                        xt["kT"][:, dt, cols],
                        w_tiles["wk"][:, dt, :],
                        start=(dt == 0),
                        stop=(dt == 1),
                    )
                for dt in range(2):
                    nc.tensor.matmul(
                        kv[:, 1, :],
                        xt["vT"][:, dt, cols],
                        w_tiles["wv"][:, dt, :],
                        start=(dt == 0),
                        stop=(dt == 1),
                    )
                nc.scalar.activation(p["ek8"][:, t0, :], kv[:, 0, :], Act.Exp)
                nc.vector.scalar_tensor_tensor(
                    p["ekv"][:, t0, :],
                    kv[:, 1, :],
                    1.0,
                    p["ek8"][:, t0, :],
                    Alu.mult,
                    Alu.mult,
                )

            def qproj_piece(p, xt, i):
                """q projection + tanh for (d_chunk, q_half) = divmod(i, 2)."""
                c, h = divmod(i, 2)
                hs = slice(h * 512, (h + 1) * 512)
                qp = qpp.tile([128, 512], F32, tag="qp")
                for j in range(2):
                    nc.tensor.matmul(
                        qp[:],
                        w_tiles["wq"][:, j, c * 128 : (c + 1) * 128],
                        xt["qT"][:, j, hs],
                        start=(j == 0),
                        stop=(j == 1),
                    )
                nc.scalar.activation(
                    p["tanh"][:, c, hs], qp[:], Act.Tanh,
                    bias=bq2_t[:, c : c + 1], scale=0.5,
                )

            def expa_chunk(p, dists, c):
                nc.scalar.activation(
                    p["expA"][:, c * 2 : (c + 1) * 2, :],
                    dists[c][:],
                    Act.Exp,
                    scale=-c_coef,
                )

            def main_unit(bi, p, i, out_t):
                """num/den/post for (d_chunk, q_half) = divmod(i, 2)."""
                c, h = divmod(i, 2)
                cs = slice(c * 128, (c + 1) * 128)
                hs = slice(h * 512, (h + 1) * 512)
                dn = denp.tile([128, 512], F32, tag="dn")
                for k in range(KT):
                    nc.tensor.matmul(
                        dn[:],
                        p["ek8"][:, k, cs],
                        p["expA"][:, k, hs],
                        start=(k == 0),
                        stop=(k == KT - 1),
                    )
                nm = nump.tile([128, 512], F32, tag="nm")
                for k in range(KT):
                    nc.tensor.matmul(
                        nm[:],
                        p["ekv"][:, k, cs],
                        p["expA"][:, k, hs],
                        start=(k == 0),
                        stop=(k == KT - 1),
                    )
                r = eps_p.tile([128, 512], F32, tag="r")
                nc.vector.reciprocal_approx_fast(r[:], dn[:])
                m = eps_p.tile([128, 512], F32, tag="m")
                a = eps_p.tile([128, 512], F32, tag="a")
                for u in range(2):
                    us = slice(u * 256, (u + 1) * 256)
                    uo = slice(h * 512 + u * 256, h * 512 + (u + 1) * 256)
                    nc.vector.tensor_mul(m[:, us], nm[:, us], r[:, us])
                    nc.vector.tensor_scalar_add(
                        a[:, us], m[:, us], bv2_t[:, c : c + 1]
                    )
                    nc.vector.scalar_tensor_tensor(
                        out_t[:, c, uo], p["tanh"][:, c, uo], 1.0, a[:, us],
                        Alu.add, Alu.mult,
                    )

            def phase_main(bi, p, nxt):
                """main(b) interleaved with stage-prep(b+1)."""
                p2 = alloc_stage() if nxt else None
                out_t = out_p.tile([128, 2, N], BF16, tag="outst")
                for i in range(4):
                    main_unit(bi, p, i, out_t)
                    if nxt is not None:
                        proj_tile(p2, nxt[0], 2 * i)
                        proj_tile(p2, nxt[0], 2 * i + 1)
                        qproj_piece(p2, nxt[0], i)
                        if i < 3:
                            expa_chunk(p2, nxt[1], i)
                    # store each finished half right away: shorter exit tail
                    c, h = divmod(i, 2)
                    nc.gpsimd.dma_start(
                        out_d[bi * 2 + c][:, h * 512 : (h + 1) * 512],
                        out_t[:, c, h * 512 : (h + 1) * 512],
                    )
                if nxt is not None:
                    expa_chunk(p2, nxt[1], 3)
                return p2

            # ---- prologue: batch 0 ----
            xt0, dists0 = phase_load(0, first=True)
            staged = alloc_stage()
            for t0 in range(KT):
                proj_tile(staged, xt0, t0)
                warm(4, cols=128)
                if t0 == 0:
                    expa_chunk(staged, dists0, 0)
                    expa_chunk(staged, dists0, 1)
                elif t0 == 2:
                    expa_chunk(staged, dists0, 2)
                elif t0 == 4:
                    expa_chunk(staged, dists0, 3)
            for i in range(4):
                qproj_piece(staged, xt0, i)
            for bi in range(BPC):
                ld = phase_load(bi + 1) if bi + 1 < BPC else None
                staged = phase_main(bi, staged, ld)

    nc.compile()
    return nc


def prepare_in_maps(inputs: dict):
    query = np.asarray(inputs["query"], dtype=np.float32)
    key_ = np.asarray(inputs["key_"], dtype=np.float32)
    value = np.asarray(inputs["value"], dtype=np.float32)
    dist = np.asarray(inputs["dist"], dtype=np.float32)
    Wq = np.asarray(inputs["Wq"], dtype=np.float32)
    Wk = np.asarray(inputs["Wk"], dtype=np.float32)
    Wv = np.asarray(inputs["Wv"], dtype=np.float32)
    bq = np.asarray(inputs["bq"], dtype=np.float32)
    bv = np.asarray(inputs["bv"], dtype=np.float32)
    alpha_raw = np.asarray(inputs["alpha_raw"], dtype=np.float64)

    alpha = float(np.logaddexp(0.0, alpha_raw)) + 1e-6  # softplus + eps
    c_coef = float(alpha * np.log2(float(N)))

    bf = ml_dtypes.bfloat16
    distT = np.ascontiguousarray(dist.transpose(0, 2, 1)).astype(bf)
    qT = np.ascontiguousarray(query.transpose(0, 2, 1)).astype(bf)
    kT = np.ascontiguousarray(key_.transpose(0, 2, 1)).astype(bf)
    vT = np.ascontiguousarray(value.transpose(0, 2, 1)).astype(bf)
    WqT = np.ascontiguousarray(Wq.T).astype(bf)
    WkT = np.ascontiguousarray(Wk.T).astype(bf)
    WvT = (np.ascontiguousarray(Wv.T) * 0.5).astype(bf)  # fold sigmoid's 0.5
    # per-partition bias columns: [128 d, 2 chunks]; 0.5 folds sigmoid scale
    bq2 = np.ascontiguousarray((0.5 * bq).reshape(2, 128).T).astype(np.float32)
    bv2 = np.ascontiguousarray((0.5 * bv).reshape(2, 128).T).astype(np.float32)

    in_maps = []
    for i in range(N_CORES):
        s = slice(i * BPC, (i + 1) * BPC)
        in_maps.append(
            {
                "distT": distT[s],
                "qT": qT[s],
                "kT": kT[s],
                "vT": vT[s],
                "WqT": WqT,
                "WkT": WkT,
                "WvT": WvT,
                "bq2": bq2,
                "bv2": bv2,
            }
        )
    return in_maps, c_coef


def run_sharded(inputs: dict, trace: bool = False):
    """Returns (full_output [32,1024,256] f32, BassKernelResults)."""
    in_maps, c_coef = prepare_in_maps(inputs)
    nc = build_graph(c_coef)
    res = run_bass_kernel_spmd(
        nc, in_maps, core_ids=list(range(N_CORES)), trace=trace
    )
    # out: per core [BPC*2, 128, N] -> [BPC, 2, 128, N] -> [BPC, N, 256]
    parts = []
    for i in range(N_CORES):
        o = np.asarray(res.results[i]["out"]).astype(np.float32)
        o = o.reshape(BPC, 2, 128, N).reshape(BPC, D, N).transpose(0, 2, 1)
        parts.append(o)
    out = np.ascontiguousarray(np.concatenate(parts, axis=0))
    return out, res


def kernel(**inputs) -> np.ndarray:
    try:
        out, _ = run_sharded(inputs, trace=False)
    except Exception:
        # one retry: transient NRT device errors (wedged core) recover on rerun
        out, _ = run_sharded(inputs, trace=False)
    return out


# revision 7
# speedup vs baseline: 1.0409x; 1.0166x over previous
"""Trainium2 Bass kernel v2 for nn_AAFM (sparse attention with distance decay).

Math (per batch b):
    q = query @ Wq.T + bq ; k = key @ Wk.T (bk cancels) ; v = value @ Wv.T
    expA = exp(-alpha*log2(N)*dist)             [n_q, n_k]
    num  = expA @ (exp(k) * v)                  [n_q, d]
    den  = expA @ exp(k)                        [n_q, d]
    out  = sigmoid(q) * (num/den + bv)

Design (v2, ~105us vs 135us v1 baseline):
  - TRANSPOSED main matmul: out^T[d, q] = (ekv|ek stationary).T @ expA moving.
    Stationaries are [128,128] slices reused across q-halves and LDWEIGHTS
    fully hides behind 512-col streams: MMs run at ~215-226ns (vs 258-286 in
    the [q,d]-oriented v1 where the expA stationary changed every MM).
  - All-bf16 datapath. fp8 (incl. DoubleRow den) was tried and REJECTED: the
    output is a weighted mean of near-zero-mean v, so any iid per-element
    perturbation of the weight chain (expA, ek, or k-proj inputs) hits the
    output at the full per-element quantization rms (~2% for e4m3) -- no
    sqrt(n) averaging, no num/den cancellation (only per-d column factors
    cancel, e.g. bk).
  - Host pre-casts dist/q/k/v to bf16: HBM traffic 30MB -> 16MB per core
    (memory floor ~45us, well under the ~85us PE floor; no DGE casts needed,
    so all steady-state loads ride the cheap sync ring, one descriptor per
    tensor -- ring sequencers pay ~0.7us per dma_start).
  - bq applied as per-partition ACT bias inside the tanh (d is the partition
    dim in the transposed layout): kills v1's separate bias pass. bv added
    per-partition on DVE via tensor_scalar_add (NEVER gpsimd tensor_scalar:
    measured 7.5us per [128,512] op there).
  - sigmoid(q) = 0.5*(tanh(q/2)+1), 0.5 folded into Wv/bv host-side (tanh
    shares the exp ACT table set; sigmoid would force ~1.5us table reloads).
  - expA produced by ACT exp directly from bf16 dist tiles (bf16-in runs
    ~1.9us per 256K elems vs 2.3us f32-in).
  - Output stored transposed [d, q] bf16; host re-transposes + upcasts.
  - Data-parallel over batch: 32 batches -> 8 cores x 4, no collectives.
  - Prologue: wk/kT/wv/vT ride the gpsimd DGE ring (issues from t=0; sync
    ring needs ~6us queue bootstrap); PE warm-up MMs bridge the DMA lead-in
    so the HAM clock governor holds 2.4GHz into the first projections.
"""

import sys

for _p in ("/opt/trn_rl_repo",):
    if _p not in sys.path:
        sys.path.append(_p)

import ml_dtypes
import numpy as np

import concourse.bass as bass
import concourse.tile as tile
from concourse import bacc, mybir
from concourse.bass_utils import run_bass_kernel_spmd

N_CORES = 8
B = 32
N = 1024  # graph size
D = 256  # d_model
BPC = B // N_CORES  # batches per core
KT = N // 128  # 8 k-tiles of 128 tokens
F32 = mybir.dt.float32
BF16 = mybir.dt.bfloat16
Alu = mybir.AluOpType
Act = mybir.ActivationFunctionType


def build_graph(c_coef: float):
    nc = bacc.Bacc(
        "TRN2", target_bir_lowering=False, debug=False, num_devices=N_CORES
    )

    distT = nc.declare_dram_parameter("distT", [BPC, N, N], BF16, isOutput=False)
    qT = nc.declare_dram_parameter("qT", [BPC, D, N], BF16, isOutput=False)
    kT = nc.declare_dram_parameter("kT", [BPC, D, N], BF16, isOutput=False)
    vT = nc.declare_dram_parameter("vT", [BPC, D, N], BF16, isOutput=False)
    WqT = nc.declare_dram_parameter("WqT", [D, D], BF16, isOutput=False)
    WkT = nc.declare_dram_parameter("WkT", [D, D], BF16, isOutput=False)
    WvT = nc.declare_dram_parameter("WvT", [D, D], BF16, isOutput=False)
    bq2_d = nc.declare_dram_parameter("bq2", [128, 2], F32, isOutput=False)
    bv2_d = nc.declare_dram_parameter("bv2", [128, 2], F32, isOutput=False)
    # out^T: [bi*2 + d_chunk, 128 d, 1024 q]
    out_d = nc.declare_dram_parameter("out", [BPC * 2, 128, N], BF16, isOutput=True)

    with tile.TileContext(nc) as tc:
        with (
            tc.tile_pool(name="const", bufs=1) as const_p,
            tc.tile_pool(name="qkv", bufs=2) as qkv_p,
            tc.tile_pool(name="dist", bufs=6) as dist_p,
            tc.tile_pool(name="stage", bufs=2) as stage_p,
            tc.tile_pool(name="eps", bufs=2) as eps_p,
            tc.tile_pool(name="outst", bufs=2) as out_p,
            tc.tile_pool(name="kvp", bufs=3, space="PSUM") as kvp,
            tc.tile_pool(name="qpp", bufs=1, space="PSUM") as qpp,
            tc.tile_pool(name="nump", bufs=2, space="PSUM") as nump,
            tc.tile_pool(name="denp", bufs=2, space="PSUM") as denp,
        ):
            # ---- constants (once) ----
            # wk/wv ride the gpsimd ring: it issues from t=0 while the sync
            # ring needs ~6us of queue bootstrap; the first kv-proj only
            # needs wk/wv + kT/vT.
            w_tiles = {}
            for nm, wd in (("wk", WkT), ("wv", WvT), ("wq", WqT)):
                w_tiles[nm] = const_p.tile([128, 2, D], BF16, tag=nm, name=nm)
            nc.gpsimd.dma_start(
                w_tiles["wk"][:], WkT[:].rearrange("(j p) e -> p j e", p=128)
            )
            nc.gpsimd.dma_start(
                w_tiles["wv"][:], WvT[:].rearrange("(j p) e -> p j e", p=128)
            )
            nc.sync.dma_start(
                w_tiles["wq"][:], WqT[:].rearrange("(j p) e -> p j e", p=128)
            )
            bq2_t = const_p.tile([128, 2], F32, tag="bq2")
            nc.sync.dma_start(bq2_t[:], bq2_d[:])
            bv2_t = const_p.tile([128, 2], F32, tag="bv2")
            nc.sync.dma_start(bv2_t[:], bv2_d[:])
            # PE warm-up while first DMAs land (HAM clock governor)
            warm_sb = const_p.tile([128, 512], BF16, tag="warm")
            nc.vector.memset(warm_sb[:], 0.0)
            warm_ps = qpp.tile([128, 512], F32, tag="qp")

            def warm(n, cols=512):
                for _ in range(n):
                    nc.tensor.matmul(
                        warm_ps[:, 0:cols], warm_sb[:, 0:128],
                        warm_sb[:, 0:cols], start=True, stop=True,
                    )

            # fill the full DMA lead-in (~8us): a >3.4us PE-idle gap lets
            # the HAM governor re-throttle and the first projections then run
            # at 1.2GHz (MM-256 max ~394ns observed). These occupy otherwise
            # idle time.
            warm(10)
            warm(44, cols=128)

            def phase_load(bi, first=False):
                # One descriptor per tensor (ring sequencers pay ~0.7us per
                # dma_start). Batch 0: kT/vT ride the gpsimd ring (starts at
                # t=0); everything else syncs in priority order.
                xt = {}
                if first:
                    for nm, xd in (("kT", kT), ("vT", vT)):
                        t = qkv_p.tile([128, 2, N], BF16, tag=nm)
                        nc.sync.dma_start(
                            t[:], xd[bi].rearrange("(j p) n -> p j n", p=128)
                        )
                        xt[nm] = t
                    dists = []
                    # first two dist quarters outrank qT on the sync ring:
                    # the serial ACT exp chain gates main(0)
                    for c in range(2):
                        dt_t = dist_p.tile([128, 2, N], BF16, tag="dist")
                        nc.sync.dma_start(
                            dt_t[:],
                            distT[bi, c * 256 : (c + 1) * 256, :].rearrange(
                                "(j p) q -> p j q", p=128
                            ),
                        )
                        dists.append(dt_t)
                    t = qkv_p.tile([128, 2, N], BF16, tag="qT")
                    nc.sync.dma_start(
                        t[:], qT[bi].rearrange("(j p) n -> p j n", p=128)
                    )
                    xt["qT"] = t
                    for c in range(2, 4):
                        dt_t = dist_p.tile([128, 2, N], BF16, tag="dist")
                        nc.sync.dma_start(
                            dt_t[:],
                            distT[bi, c * 256 : (c + 1) * 256, :].rearrange(
                                "(j p) q -> p j q", p=128
                            ),
                        )
                        dists.append(dt_t)
                    return xt, dists
                for nm, xd in (("kT", kT), ("vT", vT), ("qT", qT)):
                    t = qkv_p.tile([128, 2, N], BF16, tag=nm)
                    nc.sync.dma_start(
                        t[:], xd[bi].rearrange("(j p) n -> p j n", p=128)
                    )
                    xt[nm] = t
                dists = []
                for c in range(4):
                    dt_t = dist_p.tile([128, 2, N], BF16, tag="dist")
                    nc.sync.dma_start(
                        dt_t[:],
                        distT[bi, c * 256 : (c + 1) * 256, :].rearrange(
                            "(j p) q -> p j q", p=128
                        ),
                    )
                    dists.append(dt_t)
                return xt, dists

            def alloc_stage():
                return {
                    "expA": stage_p.tile([128, KT, N], BF16, tag="expA",
                                         name="expA"),
                    "ekv": stage_p.tile([128, KT, D], BF16, tag="ekv",
                                        name="ekv"),
                    "ek8": stage_p.tile([128, KT, D], BF16, tag="ek8",
                                        name="ek8"),
                    "tanh": stage_p.tile([128, 2, N], BF16, tag="tanh",
                                         name="tanh_t"),
                }

            def proj_tile(p, xt, t0):
                """kv projections + ek/ekv for one 128-token tile."""
                cols = slice(t0 * 128, (t0 + 1) * 128)
                kv = kvp.tile([128, 2, D], F32, tag="kv")
                for dt in range(2):
                    nc.tensor.matmul(
                        kv[:, 0, :],
                        xt["kT"][:, dt, cols],
                        w_tiles["wk"][:, dt, :],
                        start=(dt == 0),
                        stop=(dt == 1),
                    )
                for dt in range(2):
                    nc.tensor.matmul(
                        kv[:, 1, :],
                        xt["vT"][:, dt, cols],
                        w_tiles["wv"][:, dt, :],
                        start=(dt == 0),
                        stop=(dt == 1),
                    )
                nc.scalar.activation(p["ek8"][:, t0, :], kv[:, 0, :], Act.Exp)
                nc.vector.scalar_tensor_tensor(
                    p["ekv"][:, t0, :],
                    kv[:, 1, :],
                    1.0,
                    p["ek8"][:, t0, :],
                    Alu.mult,
                    Alu.mult,
                )

            def qproj_piece(p, xt, i):
                """q projection + tanh for (d_chunk, q_half) = divmod(i, 2)."""
                c, h = divmod(i, 2)
                hs = slice(h * 512, (h + 1) * 512)
                qp = qpp.tile([128, 512], F32, tag="qp")
                for j in range(2):
                    nc.tensor.matmul(
                        qp[:],
                        w_tiles["wq"][:, j, c * 128 : (c + 1) * 128],
                        xt["qT"][:, j, hs],
                        start=(j == 0),
                        stop=(j == 1),
                    )
                nc.scalar.activation(
                    p["tanh"][:, c, hs], qp[:], Act.Tanh,
                    bias=bq2_t[:, c : c + 1], scale=0.5,
                )

            def expa_chunk(p, dists, c):
                nc.scalar.activation(
                    p["expA"][:, c * 2 : (c + 1) * 2, :],
                    dists[c][:],
                    Act.Exp,
                    scale=-c_coef,
                )

            def main_unit(bi, p, i, out_t):
                """num/den/post for (d_chunk, q_half) = divmod(i, 2)."""
                c, h = divmod(i, 2)
                cs = slice(c * 128, (c + 1) * 128)
                hs = slice(h * 512, (h + 1) * 512)
                dn = denp.tile([128, 512], F32, tag="dn")
                for k in range(KT):
                    nc.tensor.matmul(
                        dn[:],
                        p["ek8"][:, k, cs],
                        p["expA"][:, k, hs],
                        start=(k == 0),
                        stop=(k == KT - 1),
                    )
                nm = nump.tile([128, 512], F32, tag="nm")
                for k in range(KT):
                    nc.tensor.matmul(
                        nm[:],
                        p["ekv"][:, k, cs],
                        p["expA"][:, k, hs],
                        start=(k == 0),
                        stop=(k == KT - 1),
                    )
                r = eps_p.tile([128, 512], F32, tag="r")
                nc.vector.reciprocal_approx_fast(r[:], dn[:])
                m = eps_p.tile([128, 512], F32, tag="m")
                a = eps_p.tile([128, 512], F32, tag="a")
                for u in range(2):
                    us = slice(u * 256, (u + 1) * 256)
                    uo = slice(h * 512 + u * 256, h * 512 + (u + 1) * 256)
                    nc.vector.tensor_mul(m[:, us], nm[:, us], r[:, us])
                    nc.vector.tensor_scalar_add(
                        a[:, us], m[:, us], bv2_t[:, c : c + 1]
                    )
                    nc.vector.scalar_tensor_tensor(
                        out_t[:, c, uo], p["tanh"][:, c, uo], 1.0, a[:, us],
                        Alu.add, Alu.mult,
                    )

            def phase_main(bi, p, nxt):
                """main(b) interleaved with stage-prep(b+1)."""
                p2 = alloc_stage() if nxt else None
                out_t = out_p.tile([128, 2, N], BF16, tag="outst")
                for i in range(4):
                    main_unit(bi, p, i, out_t)
                    if nxt is not None:
                        proj_tile(p2, nxt[0], 2 * i)
                        proj_tile(p2, nxt[0], 2 * i + 1)
                        qproj_piece(p2, nxt[0], i)
                        if i < 3:
                            expa_chunk(p2, nxt[1], i)
                    # store each finished half right away: shorter exit tail
                    c, h = divmod(i, 2)
                    nc.gpsimd.dma_start(
                        out_d[bi * 2 + c][:, h * 512 : (h + 1) * 512],
                        out_t[:, c, h * 512 : (h + 1) * 512],
                    )
                if nxt is not None:
                    expa_chunk(p2, nxt[1], 3)
                return p2

            # ---- prologue: batch 0 ----
            xt0, dists0 = phase_load(0, first=True)
            staged = alloc_stage()
            for t0 in range(KT):
                proj_tile(staged, xt0, t0)
                warm(4, cols=128)
                if t0 == 0:
                    expa_chunk(staged, dists0, 0)
                    expa_chunk(staged, dists0, 1)
                elif t0 == 2:
                    expa_chunk(staged, dists0, 2)
                elif t0 == 4:
                    expa_chunk(staged, dists0, 3)
            for i in range(4):
                qproj_piece(staged, xt0, i)
            for bi in range(BPC):
                ld = phase_load(bi + 1) if bi + 1 < BPC else None
                staged = phase_main(bi, staged, ld)

    nc.compile()
    return nc


def prepare_in_maps(inputs: dict):
    query = np.asarray(inputs["query"], dtype=np.float32)
    key_ = np.asarray(inputs["key_"], dtype=np.float32)
    value = np.asarray(inputs["value"], dtype=np.float32)
    dist = np.asarray(inputs["dist"], dtype=np.float32)
    Wq = np.asarray(inputs["Wq"], dtype=np.float32)
    Wk = np.asarray(inputs["Wk"], dtype=np.float32)
    Wv = np.asarray(inputs["Wv"], dtype=np.float32)
    bq = np.asarray(inputs["bq"], dtype=np.float32)
    bv = np.asarray(inputs["bv"], dtype=np.float32)
    alpha_raw = np.asarray(inputs["alpha_raw"], dtype=np.float64)

    alpha = float(np.logaddexp(0.0, alpha_raw)) + 1e-6  # softplus + eps
    c_coef = float(alpha * np.log2(float(N)))

    bf = ml_dtypes.bfloat16
    distT = np.ascontiguousarray(dist.transpose(0, 2, 1)).astype(bf)
    qT = np.ascontiguousarray(query.transpose(0, 2, 1)).astype(bf)
    kT = np.ascontiguousarray(key_.transpose(0, 2, 1)).astype(bf)
    vT = np.ascontiguousarray(value.transpose(0, 2, 1)).astype(bf)
    WqT = np.ascontiguousarray(Wq.T).astype(bf)
    WkT = np.ascontiguousarray(Wk.T).astype(bf)
    WvT = (np.ascontiguousarray(Wv.T) * 0.5).astype(bf)  # fold sigmoid's 0.5
    # per-partition bias columns: [128 d, 2 chunks]; 0.5 folds sigmoid scale
    bq2 = np.ascontiguousarray((0.5 * bq).reshape(2, 128).T).astype(np.float32)
    bv2 = np.ascontiguousarray((0.5 * bv).reshape(2, 128).T).astype(np.float32)

    in_maps = []
    for i in range(N_CORES):
        s = slice(i * BPC, (i + 1) * BPC)
        in_maps.append(
            {
                "distT": distT[s],
                "qT": qT[s],
                "kT": kT[s],
                "vT": vT[s],
                "WqT": WqT,
                "WkT": WkT,
                "WvT": WvT,
                "bq2": bq2,
                "bv2": bv2,
            }
        )
    return in_maps, c_coef


def run_sharded(inputs: dict, trace: bool = False):
    """Returns (full_output [32,1024,256] f32, BassKernelResults)."""
    in_maps, c_coef = prepare_in_maps(inputs)
    nc = build_graph(c_coef)
    res = run_bass_kernel_spmd(
        nc, in_maps, core_ids=list(range(N_CORES)), trace=trace
    )
    # out: per core [BPC*2, 128, N] -> [BPC, 2, 128, N] -> [BPC, N, 256]
    parts = []
    for i in range(N_CORES):
        o = np.asarray(res.results[i]["out"]).astype(np.float32)
        o = o.reshape(BPC, 2, 128, N).reshape(BPC, D, N).transpose(0, 2, 1)
        parts.append(o)
    out = np.ascontiguousarray(np.concatenate(parts, axis=0))
    return out, res


def kernel(**inputs) -> np.ndarray:
    try:
        out, _ = run_sharded(inputs, trace=False)
    except Exception:
        # one retry: transient NRT device errors (wedged core) recover on rerun
        out, _ = run_sharded(inputs, trace=False)
    return out
